# revision 1
# baseline (speedup 1.0000x reference)
"""MLA (multi-head latent attention) forward, sharded over 8 TRN2 NeuronCores.

Tensor-parallel over heads (2 heads/core).  Host folds rmsnorm weights into
the B-projections and fuses A@B into per-head projection weights, exploiting
that rmsnorm's per-token scale commutes through the matmul:
    rmsnorm(x) @ Wb.T == (x @ (Wb*w).T) / rms(x)
Phase 0 computes inv_rms for each core's 512-token shard; one tiny AllGather
shares the 4096+4096 scalars.  Phase 1 produces qT/kT (feature-major, roped,
scaled) + v (token-major) for the core's heads in one pass over hidden^T.
Phase 2 runs attention per (head, batch, 512-query block) with scores^T kept
in PSUM, exp on the scalar engine, denominator via a ones-matmul.  Phase 3
applies the per-head slice of wo; the host sums the 8 partial outputs (the
"all-reduce after wo" of the sharding hint, free on host).

Matmuls run in bf16 with fp32 PSUM accumulation; softmax and statistics stay
fp32.  RoPE features are permuted to a split even/odd layout on both q and k
(host permutes the projection weight rows identically, dot products unchanged).
"""
import sys

sys.path.insert(0, "/opt/trn_rl_repo")

import numpy as np
import ml_dtypes

import concourse.mybir as mybir
from concourse import bacc
from concourse.tile import TileContext
from concourse.bass_utils import run_bass_kernel_spmd

BF16 = ml_dtypes.bfloat16
F32 = mybir.dt.float32
BF = mybir.dt.bfloat16

B, S, H = 2, 2048, 2048
NH = 16
Q_LORA, KV_LORA = 1536, 512
D_NOPE, D_ROPE, D_V = 128, 64, 128
D_QK = D_NOPE + D_ROPE
SCALE = 1.0 / float(np.sqrt(D_QK))
EPS = 1e-6

N_CORES = 8
HPC = NH // N_CORES          # heads per core = 2
TOK = B * S                  # 4096
TOKS = TOK // N_CORES        # 512-token rms shard per core
KC = H // 128                # 16 contraction chunks over hidden features
NB = TOK // 512              # 8 token blocks of 512

# W_all column layout (projection output features, per core):
#   [0:128)   q_nope h0   [128:256) q_nope h1
#   [256:384) q_pe stack: E_h0(32) E_h1(32) O_h0(32) O_h1(32)
#   [384:512) k_nope h0   [512:640) k_nope h1
#   [640:704) k_pe stack: E(32) O(32)
#   [704:960) v h0(128) v h1(128)
NPROJ = 960


def _host_tables():
    inv = 1.0 / (10000.0 ** (np.arange(0, D_ROPE, 2, dtype=np.float32) / D_ROPE))
    t = np.arange(S, dtype=np.float32)
    f = np.outer(t, inv)                       # (S, 32)
    cos = np.tile(np.cos(f).T, (1, B))         # (32, TOK), tokens b-major
    sin = np.tile(np.sin(f).T, (1, B))
    csq1 = np.concatenate([cos, cos, sin, sin], axis=0)   # (128, TOK)
    csq2 = np.concatenate([sin, sin, cos, cos], axis=0)
    csk1 = np.concatenate([cos, sin], axis=0)             # (64, TOK)
    csk2 = np.concatenate([sin, cos], axis=0)
    return [np.ascontiguousarray(x).astype(BF16) for x in (csq1, csq2, csk1, csk2)]


def _host_prep(hidden_states, wq_a, q_norm_w, wq_b, wkv_a, kv_norm_w, wkv_b, wo):
    hid = np.ascontiguousarray(np.asarray(hidden_states, dtype=np.float32).reshape(TOK, H))
    hT_bf = np.ascontiguousarray(hid.T).astype(BF16)             # (H, TOK)

    wq_b_f = (np.asarray(wq_b) * np.asarray(q_norm_w)[None, :]).astype(np.float32)
    wkv_b_f = (np.asarray(wkv_b) * np.asarray(kv_norm_w)[None, :]).astype(np.float32)

    Wq = wq_b_f @ np.asarray(wq_a)                 # (NH*192, H)
    Wkv = wkv_b_f @ np.asarray(wkv_a)[:KV_LORA]    # (NH*256, H)
    wkpe = np.asarray(wkv_a)[KV_LORA:]             # (64, H)

    ev = np.arange(0, D_ROPE, 2)
    od = np.arange(1, D_ROPE, 2)
    csq1, csq2, csk1, csk2 = _host_tables()

    wqaT = np.ascontiguousarray(np.asarray(wq_a).T).astype(BF16)
    wkvaT = np.ascontiguousarray(np.asarray(wkv_a)[:KV_LORA].T).astype(BF16)

    in_maps = []
    for c in range(N_CORES):
        h0, h1 = 2 * c, 2 * c + 1
        qh = [Wq[h * D_QK:(h + 1) * D_QK] for h in (h0, h1)]
        kvh = [Wkv[h * (D_NOPE + D_V):(h + 1) * (D_NOPE + D_V)] for h in (h0, h1)]
        qpe0, qpe1 = qh[0][D_NOPE:], qh[1][D_NOPE:]
        W_all = np.concatenate([
            qh[0][:D_NOPE], qh[1][:D_NOPE],
            qpe0[ev], qpe1[ev], qpe0[od], qpe1[od],
            kvh[0][:D_NOPE], kvh[1][:D_NOPE],
            wkpe[ev], wkpe[od],
            kvh[0][D_NOPE:], kvh[1][D_NOPE:],
        ], axis=0)                                               # (960, H)
        WallT = np.ascontiguousarray(W_all.T).astype(BF16)       # (H, 960)
        wo_h = np.asarray(wo)[:, c * HPC * D_V:(c + 1) * HPC * D_V]   # (H, 256)
        woR = np.ascontiguousarray(wo_h.T).astype(BF16)          # (256, H)

        in_maps.append({
            "hT": hT_bf,
            "hTs": np.ascontiguousarray(hT_bf[:, c * TOKS:(c + 1) * TOKS]),
            "wqaT": wqaT,
            "wkvaT": wkvaT,
            "WallT": WallT,
            "woR": woR,
            "csq1": csq1, "csq2": csq2, "csk1": csk1, "csk2": csk2,
        })
    return in_maps


def _build_program():
    nc = bacc.Bacc()

    hT = nc.dram_tensor("hT", [H, TOK], BF, kind="ExternalInput")
    hTs = nc.dram_tensor("hTs", [H, TOKS], BF, kind="ExternalInput")
    wqaT = nc.dram_tensor("wqaT", [H, Q_LORA], BF, kind="ExternalInput")
    wkvaT = nc.dram_tensor("wkvaT", [H, KV_LORA], BF, kind="ExternalInput")
    WallT = nc.dram_tensor("WallT", [H, NPROJ], BF, kind="ExternalInput")
    woR = nc.dram_tensor("woR", [HPC * D_V, H], BF, kind="ExternalInput")
    csq1d = nc.dram_tensor("csq1", [128, TOK], BF, kind="ExternalInput")
    csq2d = nc.dram_tensor("csq2", [128, TOK], BF, kind="ExternalInput")
    csk1d = nc.dram_tensor("csk1", [64, TOK], BF, kind="ExternalInput")
    csk2d = nc.dram_tensor("csk2", [64, TOK], BF, kind="ExternalInput")
    out = nc.dram_tensor("out", [TOK, H], F32, kind="ExternalOutput")

    AF = mybir.ActivationFunctionType
    OP = mybir.AluOpType

    with TileContext(nc) as tc:
        with tc.tile_pool(name="invp", bufs=1) as invp, \
             tc.tile_pool(name="onesp", bufs=1) as onesp, \
             tc.tile_pool(name="ccp", bufs=1, space="DRAM") as ccp:

            ones_col = onesp.tile([128, 1], F32)
            ones_row = onesp.tile([1, 128], F32)
            eps_col = onesp.tile([128, 1], F32)
            nc.vector.memset(ones_col[:], 1.0)
            nc.vector.memset(ones_row[:], 1.0)
            nc.vector.memset(eps_col[:], EPS)
            cc_in = ccp.tile([1, 2 * TOKS], F32)
            cc_out = ccp.tile([N_CORES, 2 * TOKS], F32)

            # ---------------- phase 0: rms scalars for own token shard ----
            with tc.tile_pool(name="p0w", bufs=1) as p0w, \
                 tc.tile_pool(name="p0ps", bufs=2, space="PSUM") as p0ps, \
                 tc.tile_pool(name="p0sb", bufs=2) as p0sb:

                hts_t = []
                for k in range(KC):
                    t = p0w.tile([128, TOKS], BF, tag=f"hts{k}", name=f"hts{k}")
                    nc.gpsimd.dma_start(t[:], hTs[k * 128:(k + 1) * 128, :])
                    hts_t.append(t)
                wqa_t = []
                for k in range(KC):
                    t = p0w.tile([128, Q_LORA], BF, tag=f"wqa{k}", name=f"wqa{k}")
                    nc.gpsimd.dma_start(t[:], wqaT[k * 128:(k + 1) * 128, :])
                    wqa_t.append(t)
                wkva_t = []
                for k in range(KC):
                    t = p0w.tile([128, KV_LORA], BF, tag=f"wkva{k}", name=f"wkva{k}")
                    nc.gpsimd.dma_start(t[:], wkvaT[k * 128:(k + 1) * 128, :])
                    wkva_t.append(t)

                for tb in range(TOKS // 128):        # 4 blocks of 128 tokens
                    tsl = slice(tb * 128, (tb + 1) * 128)
                    ss_q = p0sb.tile([128, 1], F32, tag="ssq")
                    ss_kv = p0sb.tile([128, 1], F32, tag="sskv")
                    scratch = p0sb.tile([128, 512], F32, tag="scr")
                    for nb3 in range(Q_LORA // 512):
                        ps = p0ps.tile([128, 512], F32, tag="p0ps")
                        for k in range(KC):
                            nc.tensor.matmul(
                                ps[:], lhsT=hts_t[k][:, tsl],
                                rhs=wqa_t[k][:, nb3 * 512:(nb3 + 1) * 512],
                                start=(k == 0), stop=(k == KC - 1))
                        ssp = p0sb.tile([128, 1], F32, tag=f"ssp{nb3}", name=f"ssp{nb3}")
                        nc.scalar.activation(scratch[:], ps[:], AF.Square,
                                             accum_out=ssp[:])
                        if nb3 == 0:
                            nc.vector.tensor_copy(ss_q[:], ssp[:])
                        else:
                            nc.vector.tensor_add(ss_q[:], ss_q[:], ssp[:])
                    ps = p0ps.tile([128, 512], F32, tag="p0ps")
                    for k in range(KC):
                        nc.tensor.matmul(
                            ps[:], lhsT=hts_t[k][:, tsl], rhs=wkva_t[k][:],
                            start=(k == 0), stop=(k == KC - 1))
                    nc.scalar.activation(scratch[:], ps[:], AF.Square,
                                         accum_out=ss_kv[:])

                    rms_q = p0sb.tile([128, 1], F32, tag="rmsq")
                    rms_kv = p0sb.tile([128, 1], F32, tag="rmskv")
                    nc.scalar.activation(rms_q[:], ss_q[:], AF.Sqrt,
                                         bias=eps_col[:], scale=1.0 / Q_LORA)
                    nc.scalar.activation(rms_kv[:], ss_kv[:], AF.Sqrt,
                                         bias=eps_col[:], scale=1.0 / KV_LORA)
                    inv_q = p0sb.tile([128, 1], F32, tag="invq")
                    inv_kv = p0sb.tile([128, 1], F32, tag="invkv")
                    nc.vector.reciprocal(inv_q[:], rms_q[:])
                    nc.vector.reciprocal(inv_kv[:], rms_kv[:])
                    nc.gpsimd.dma_start(cc_in[0, tsl], inv_q[:])
                    nc.gpsimd.dma_start(
                        cc_in[0, TOKS + tb * 128:TOKS + (tb + 1) * 128], inv_kv[:])

            nc.gpsimd.collective_compute(
                "AllGather", OP.bypass,
                replica_groups=[list(range(N_CORES))],
                ins=[cc_in.opt()], outs=[cc_out.opt()])

            # persistent activations
            with tc.tile_pool(name="acts", bufs=1) as acts, \
                 tc.tile_pool(name="invbc", bufs=1) as invbc:
                qn = [acts.tile([128, TOK], BF, tag=f"qn{h}", name=f"qn{h}") for h in range(HPC)]
                qpe = [acts.tile([64, TOK], BF, tag=f"qpe{h}", name=f"qpe{h}") for h in range(HPC)]
                kn = [acts.tile([128, TOK], BF, tag=f"kn{h}", name=f"kn{h}") for h in range(HPC)]
                kpe = acts.tile([64, TOK], BF, tag="kpe", name="kpe")
                vnat = [acts.tile([128, HPC * D_V], BF, tag=f"v{i}", name=f"v{i}")
                        for i in range(TOK // 128)]

                inv_q_bc = invbc.tile([128, TOK], F32, name="inv_q_bc")
                inv_kv_bc = invbc.tile([128, TOK], F32, name="inv_kv_bc")

                # broadcast inv_rms rows across 128 partitions (K=1 matmuls)
                with tc.tile_pool(name="rowp", bufs=1) as rowp, \
                     tc.tile_pool(name="bcps", bufs=2, space="PSUM") as bcps:
                    inv_q_row = rowp.tile([1, N_CORES, TOKS], F32, name="inv_q_row")
                    inv_kv_row = rowp.tile([1, N_CORES, TOKS], F32, name="inv_kv_row")
                    nc.gpsimd.dma_start(inv_q_row[:], cc_out[:, 0:TOKS])
                    nc.gpsimd.dma_start(inv_kv_row[:], cc_out[:, TOKS:2 * TOKS])
                    for j in range(NB):
                        sl = slice(j * 512, (j + 1) * 512)
                        psb = bcps.tile([128, 512], F32, tag="psb", name="psb")
                        nc.tensor.matmul(psb[:], lhsT=ones_row[:],
                                         rhs=inv_q_row[0:1, j, 0:TOKS],
                                         start=True, stop=True)
                        nc.vector.tensor_copy(inv_q_bc[:, sl], psb[:])
                        psb2 = bcps.tile([128, 512], F32, tag="psb", name="psb2")
                        nc.tensor.matmul(psb2[:], lhsT=ones_row[:],
                                         rhs=inv_kv_row[0:1, j, 0:TOKS],
                                         start=True, stop=True)
                        nc.vector.tensor_copy(inv_kv_bc[:, sl], psb2[:])

                # ------------- phase 1: fused projections ------------------
                with tc.tile_pool(name="p1w", bufs=1) as p1w, \
                     tc.tile_pool(name="csp", bufs=1) as csp, \
                     tc.tile_pool(name="p1h", bufs=1) as p1h, \
                     tc.tile_pool(name="p1ps", bufs=1, space="PSUM") as p1ps, \
                     tc.tile_pool(name="p1vps", bufs=1, space="PSUM") as p1vps, \
                     tc.tile_pool(name="p1sb", bufs=1) as p1sb:

                    wall_t = []
                    for k in range(KC):
                        t = p1w.tile([128, NPROJ], BF, tag=f"wall{k}", name=f"wall{k}")
                        nc.gpsimd.dma_start(t[:], WallT[k * 128:(k + 1) * 128, :])
                        wall_t.append(t)
                    csq1_t = csp.tile([128, TOK], BF, tag="csq1", name="csq1")
                    csq2_t = csp.tile([128, TOK], BF, tag="csq2", name="csq2")
                    nc.gpsimd.dma_start(csq1_t[:], csq1d[:])
                    nc.gpsimd.dma_start(csq2_t[:], csq2d[:])
                    csk1_t = csp.tile([64, TOK], BF, tag="csk1", name="csk1")
                    csk2_t = csp.tile([64, TOK], BF, tag="csk2", name="csk2")
                    nc.gpsimd.dma_start(csk1_t[:], csk1d[:])
                    nc.gpsimd.dma_start(csk2_t[:], csk2d[:])

                    for nb in range(NB):             # 8 blocks of 512 tokens
                        tsl = slice(nb * 512, (nb + 1) * 512)
                        ht_c = []
                        for k in range(KC):
                            t = p1h.tile([128, 512], BF, tag=f"htc{k}", name=f"htc{k}")
                            nc.gpsimd.dma_start(t[:], hT[k * 128:(k + 1) * 128, tsl])
                            ht_c.append(t)

                        ps_feat = [p1ps.tile([128, 512], F32, tag=f"pf{mb}", name=f"pf{mb}")
                                   for mb in range(5)]
                        ps_feat.append(p1ps.tile([64, 512], F32, tag="pf5", name="pf5"))
                        ps_v = [p1vps.tile([128, 512], F32, tag=f"pv{i}", name=f"pv{i}")
                                for i in range(2)]
                        for k in range(KC):
                            for mb in range(6):
                                mrows = 64 if mb == 5 else 128
                                nc.tensor.matmul(
                                    ps_feat[mb][:],
                                    lhsT=wall_t[k][:, mb * 128:mb * 128 + mrows],
                                    rhs=ht_c[k][:],
                                    start=(k == 0), stop=(k == KC - 1))
                            for sb4 in range(4):
                                nc.tensor.matmul(
                                    ps_v[sb4 // 2][:, (sb4 % 2) * 256:
                                                   (sb4 % 2) * 256 + 256],
                                    lhsT=ht_c[k][:, sb4 * 128:(sb4 + 1) * 128],
                                    rhs=wall_t[k][:, 704:960],
                                    start=(k == 0 and sb4 % 2 == 0),
                                    stop=(k == KC - 1 and sb4 % 2 == 1))

                        bq = inv_q_bc[:, tsl]
                        bkv = inv_kv_bc[:, tsl]
                        # evict + scale + cast
                        nc.vector.tensor_mul(qn[0][:, tsl], ps_feat[0][:], bq)
                        nc.vector.tensor_mul(qn[1][:, tsl], ps_feat[1][:], bq)
                        nc.vector.tensor_mul(kn[0][:, tsl], ps_feat[3][:], bkv)
                        nc.vector.tensor_mul(kn[1][:, tsl], ps_feat[4][:], bkv)

                        # v: per-token (partition) scale via activation scale
                        for sb4 in range(4):
                            tm = nb * 4 + sb4
                            ivc = p1sb.tile([128, 1], F32, tag="ivc", name="ivc")
                            nc.gpsimd.dma_start(
                                ivc[:],
                                cc_out[tm // 4, TOKS + (tm % 4) * 128:
                                       TOKS + (tm % 4) * 128 + 128])
                            nc.vector.tensor_scalar_mul(
                                out=vnat[tm][:],
                                in0=ps_v[sb4 // 2][:, (sb4 % 2) * 256:(sb4 % 2) * 256 + 256],
                                scalar1=ivc[:])

                        # rope q_pe stack [E0 E1 O0 O1] (scaled by inv_q)
                        tq = p1sb.tile([128, 512], F32, tag="tq", name="tq")
                        nc.vector.tensor_mul(tq[:], ps_feat[2][:], bq)
                        m1a = p1sb.tile([64, 512], F32, tag="m1a", name="m1a")
                        m1b = p1sb.tile([64, 512], F32, tag="m1b", name="m1b")
                        m2a = p1sb.tile([64, 512], F32, tag="m2a", name="m2a")
                        m2b = p1sb.tile([64, 512], F32, tag="m2b", name="m2b")
                        # tq rows: [E0 E1 O0 O1]; csq1=[C C S S], csq2=[S S C C]
                        nc.vector.tensor_mul(m1a[:], tq[0:64, :], csq1_t[0:64, tsl])
                        nc.vector.tensor_mul(m1b[:], tq[64:128, :], csq1_t[64:128, tsl])
                        nc.vector.tensor_mul(m2a[:], tq[0:64, :], csq2_t[0:64, tsl])
                        nc.vector.tensor_mul(m2b[:], tq[64:128, :], csq2_t[64:128, tsl])
                        nc.vector.tensor_sub(qpe[0][0:32, tsl], m1a[0:32, :], m1b[0:32, :])
                        nc.vector.tensor_add(qpe[0][32:64, tsl], m2a[0:32, :], m2b[0:32, :])
                        nc.vector.tensor_sub(qpe[1][0:32, tsl], m1a[32:64, :], m1b[32:64, :])
                        nc.vector.tensor_add(qpe[1][32:64, tsl], m2a[32:64, :], m2b[32:64, :])

                        # rope k_pe stack [E O]; cs rows reused:
                        #   [C;S] = csq1[32:96], [S;C] = csq2[32:96]
                        mka = p1sb.tile([32, 512], F32, tag="mka", name="mka")
                        mkb = p1sb.tile([32, 512], F32, tag="mkb", name="mkb")
                        mkc = p1sb.tile([32, 512], F32, tag="mkc", name="mkc")
                        mkd = p1sb.tile([32, 512], F32, tag="mkd", name="mkd")
                        # k stack rows: [E O]; csk1=[C S], csk2=[S C]
                        nc.vector.tensor_mul(mka[:], ps_feat[5][0:32, :], csk1_t[0:32, tsl])
                        nc.vector.tensor_mul(mkb[:], ps_feat[5][32:64, :], csk1_t[32:64, tsl])
                        nc.vector.tensor_mul(mkc[:], ps_feat[5][0:32, :], csk2_t[0:32, tsl])
                        nc.vector.tensor_mul(mkd[:], ps_feat[5][32:64, :], csk2_t[32:64, tsl])
                        nc.vector.tensor_sub(kpe[0:32, tsl], mka[:], mkb[:])
                        nc.vector.tensor_add(kpe[32:64, tsl], mkc[:], mkd[:])

                # ------------- phase 2+3: attention + wo -------------------
                with tc.tile_pool(name="wop", bufs=1) as wop, \
                     tc.tile_pool(name="sps", bufs=2, space="PSUM") as sps, \
                     tc.tile_pool(name="ops", bufs=2, space="PSUM") as ops, \
                     tc.tile_pool(name="dps", bufs=1, space="PSUM") as dps, \
                     tc.tile_pool(name="bps", bufs=1, space="PSUM") as bps, \
                     tc.tile_pool(name="wps", bufs=2, space="PSUM") as wps, \
                     tc.tile_pool(name="esb", bufs=4) as esb, \
                     tc.tile_pool(name="asb", bufs=3) as asb, \
                     tc.tile_pool(name="otp", bufs=1) as otp, \
                     tc.tile_pool(name="osb", bufs=3) as osb:

                    wo_t = []
                    for i in range(2):
                        t = wop.tile([128, H], BF, tag=f"wot{i}", name=f"wot{i}")
                        nc.gpsimd.dma_start(t[:], woR[i * 128:(i + 1) * 128, :])
                        wo_t.append(t)

                    for b in range(B):
                        outT = [otp.tile([128, S], BF, tag=f"outT{h}", name=f"outT{h}")
                                for h in range(HPC)]
                        for h in range(HPC):
                            for qb in range(S // 512):
                                qsl = slice(b * S + qb * 512, b * S + qb * 512 + 512)
                                osl = slice(qb * 512, qb * 512 + 512)
                                ps_o = ops.tile([128, 512], F32, tag="ps_o", name="ps_o")
                                acc_d = asb.tile([128, 512], F32, tag="acc_d", name="acc_d")
                                for kc in range(S // 128):
                                    ksl = slice(b * S + kc * 128,
                                                b * S + kc * 128 + 128)
                                    ps_s = sps.tile([128, 512], F32, tag="ps_s", name="ps_s")
                                    nc.tensor.matmul(ps_s[:], lhsT=kn[h][:, ksl],
                                                     rhs=qn[h][:, qsl],
                                                     start=True, stop=False)
                                    nc.tensor.matmul(
                                        ps_s[:], lhsT=kpe[:, ksl],
                                        rhs=qpe[h][:, qsl],
                                        start=False, stop=True)
                                    expT = esb.tile([128, 512], BF, tag="expT", name="expT")
                                    nc.scalar.activation(expT[:], ps_s[:], AF.Exp,
                                                         scale=SCALE)
                                    if kc == 0:
                                        nc.vector.tensor_copy(acc_d[:], expT[:])
                                    else:
                                        nc.vector.tensor_add(acc_d[:], acc_d[:],
                                                             expT[:])
                                    tm = (b * S) // 128 + kc
                                    nc.tensor.matmul(
                                        ps_o[:],
                                        lhsT=vnat[tm][:, h * D_V:(h + 1) * D_V],
                                        rhs=expT[:],
                                        start=(kc == 0), stop=(kc == S // 128 - 1))
                                # denominator -> reciprocal -> broadcast
                                ps_d = dps.tile([1, 512], F32, tag="ps_d", name="ps_d")
                                nc.tensor.matmul(ps_d[:], lhsT=ones_col[:],
                                                 rhs=acc_d[:], start=True, stop=True)
                                rec = esb.tile([1, 512], F32, tag="rec", name="rec")
                                nc.vector.reciprocal(rec[:], ps_d[:])
                                ps_bc = bps.tile([128, 512], F32, tag="ps_bc", name="ps_bc")
                                nc.tensor.matmul(ps_bc[:], lhsT=ones_row[:],
                                                 rhs=rec[:], start=True, stop=True)
                                bc_sb = esb.tile([128, 512], F32, tag="bc_sb", name="bc_sb")
                                nc.vector.tensor_copy(bc_sb[:], ps_bc[:])
                                nc.vector.tensor_mul(outT[h][:, osl], ps_o[:],
                                                     bc_sb[:])

                        # wo for this batch (overlaps with next batch attn)
                        for tmb in range(S // 128):
                            osl = slice(tmb * 128, tmb * 128 + 128)
                            trow = b * S + tmb * 128
                            for hn in range(H // 512):
                                ps_w = wps.tile([128, 512], F32, tag="ps_w", name="ps_w")
                                for h in range(HPC):
                                    nc.tensor.matmul(
                                        ps_w[:],
                                        lhsT=outT[h][:, osl],
                                        rhs=wo_t[h][:, hn * 512:(hn + 1) * 512],
                                        start=(h == 0), stop=(h == HPC - 1))
                                o_sb = osb.tile([128, 512], F32, tag="o_sb", name="o_sb")
                                nc.vector.tensor_copy(o_sb[:], ps_w[:])
                                nc.gpsimd.dma_start(
                                    out[trow:trow + 128, hn * 512:(hn + 1) * 512],
                                    o_sb[:])

    nc.compile()
    return nc


_PROGRAM = None


def _get_program():
    global _PROGRAM
    if _PROGRAM is None:
        _PROGRAM = _build_program()
    return _PROGRAM


def kernel(hidden_states, wq_a, q_norm_w, wq_b, wkv_a, kv_norm_w, wkv_b, wo):
    nc = _get_program()
    in_maps = _host_prep(hidden_states, wq_a, q_norm_w, wq_b,
                         wkv_a, kv_norm_w, wkv_b, wo)
    res = run_bass_kernel_spmd(nc, in_maps, list(range(N_CORES)))
    total = np.zeros((TOK, H), dtype=np.float32)
    for r in res.results:
        total += r["out"]
    return total.reshape(B, S, H)



# revision 24
# speedup vs baseline: 28276.6480x; 28276.6480x over previous
"""MLA forward, sharded over 8 TRN2 NeuronCores.

Tensor-parallel over heads (2/core).  Host folds rmsnorm weights into the
B-projections and fuses A@B per head (rmsnorm's per-token scale commutes:
rmsnorm(x) @ Wb.T == (x @ (Wb*w).T) / rms(x)).

Precision strategy (fp8e4m3 flat-noise on zero-mean data is ~3-5% of the
output, so fp8 is only used where the rounding is provably recovered):
  - fused projection: fp8 DoubleRow with a 3-term residual expansion
      h*W ~= h8.W8 + hr8.W8 + h8.Wr8   (hr8/Wr8 = fp8 of the fp8 rounding
    error; leftover hr.Wr term is ~0.1%).  DoubleRow packs K=256 per
    instruction at 0.5 PE cycles/row, so 3 terms still beat bf16 by 25%.
  - phase-0 rms statistics: single-term fp8 DoubleRow (rms is a mean of
    1536 squares; fp8 noise averages to <0.2% there).
  - everything else (scores, exp, PV, denominator, wo) in bf16.

Phases:
  0  rms stats for the core's 512-token shard, AllGather of inv_rms.
  1  fused projection per 512-token block -> qn/qpe/kn/kpe feature-major
     bf16, v token-major bf16.  Evictions: DVE (q/k + q-rope muls),
     Pool (rope add/sub from SBUF), ACT (v via per-partition inv scale).
  2  scores^T per 128-k-block (bf16 nope K=128 + rope K=64 into one PSUM
     bank), exp on ACT (global shift 2.0), PV bf16, denominator via
     ones-matmuls accumulated in PSUM, normalize, then bf16 wo.
  host sums the 8 bf16 partial outputs (the "all-reduce after wo").
"""
import sys

sys.path.insert(0, "/opt/trn_rl_repo")

import numpy as np
import ml_dtypes

import concourse.mybir as mybir
from concourse import bacc
from concourse.tile import TileContext
from concourse.bass_utils import run_bass_kernel_spmd

NP8 = ml_dtypes.float8_e4m3
BF16 = ml_dtypes.bfloat16
F32 = mybir.dt.float32
BF = mybir.dt.bfloat16
F8 = mybir.dt.float8e4
DR = mybir.MatmulPerfMode.DoubleRow

B, S, H = 2, 2048, 2048
NH = 16
Q_LORA, KV_LORA = 1536, 512
D_NOPE, D_ROPE, D_V = 128, 64, 128
D_QK = D_NOPE + D_ROPE
SCALE = 1.0 / float(np.sqrt(D_QK))
EPS = 1e-6

N_CORES = 8
HPC = NH // N_CORES          # heads per core = 2
TOK = B * S                  # 4096
TOKS = TOK // N_CORES        # 512-token rms shard per core
KCP = H // 256               # 8 contraction PAIRS over hidden features
NB = TOK // 512              # 8 token blocks of 512

HS = 32.0                    # hidden fp8 scale (2^5)
WS = 512.0                   # weight fp8 scale (2^9)
EPS_SC = EPS * (HS * WS) ** 2    # eps * 2^28, for scaled sum-of-squares
SHIFT = 2.0                  # global softmax exp shift (softmax-invariant)

# W_all column layout (projection output features, per core):
#   [0:128) qn h0  [128:256) qn h1  [256:384) qpe E0 E1 O0 O1 (32 each)
#   [384:512) kn h0  [512:640) kn h1  [640:704) kpe E(32) O(32)
#   [704:960) v h0(128) v h1(128)
NPROJ = 960


def _pack_contract(a):
    """(H, F) f32 -> ([128, KCP, 2, F] fp8 main, same-shape fp8 residual)."""
    hdim, f = a.shape
    assert hdim == H
    p = np.ascontiguousarray(a.reshape(KCP, 2, 128, f).transpose(2, 0, 1, 3))
    m = p.astype(NP8)
    r = (p - m.astype(np.float32)).astype(NP8)
    return m, r


def _host_tables():
    inv = 1.0 / (10000.0 ** (np.arange(0, D_ROPE, 2, dtype=np.float32) / D_ROPE))
    t = np.arange(S, dtype=np.float32)
    f = np.outer(t, inv)                       # (S, 32)
    cos = np.tile(np.cos(f).T, (1, B))         # (32, TOK), tokens b-major
    sin = np.tile(np.sin(f).T, (1, B))
    csq1 = np.concatenate([cos, cos, sin, sin], axis=0)   # (128, TOK)
    csq2 = np.concatenate([sin, sin, cos, cos], axis=0)
    kd = 1.0 / (HS * WS)                       # 2^-14 descale for k_pe
    csk1 = np.concatenate([cos, sin], axis=0) * kd        # (64, TOK)
    csk2 = np.concatenate([sin, cos], axis=0) * kd
    return [np.ascontiguousarray(x).astype(BF16) for x in (csq1, csq2, csk1, csk2)]


def _host_prep(hidden_states, wq_a, q_norm_w, wq_b, wkv_a, kv_norm_w, wkv_b, wo):
    hid = np.ascontiguousarray(
        np.asarray(hidden_states, dtype=np.float32).reshape(TOK, H))
    hT8, hTr8 = _pack_contract(np.ascontiguousarray(hid.T) * HS)

    wq_b_f = (np.asarray(wq_b) * np.asarray(q_norm_w)[None, :]).astype(np.float32)
    wkv_b_f = (np.asarray(wkv_b) * np.asarray(kv_norm_w)[None, :]).astype(np.float32)

    Wq = wq_b_f @ np.asarray(wq_a)                 # (NH*192, H)
    Wkv = wkv_b_f @ np.asarray(wkv_a)[:KV_LORA]    # (NH*256, H)
    wkpe = np.asarray(wkv_a)[KV_LORA:]             # (64, H)

    ev = np.arange(0, D_ROPE, 2)
    od = np.arange(1, D_ROPE, 2)
    csq1, csq2, csk1, csk2 = _host_tables()

    wqa8, _ = _pack_contract(np.asarray(wq_a).T.astype(np.float32) * WS)
    wkva8, _ = _pack_contract(np.asarray(wkv_a)[:KV_LORA].T.astype(np.float32) * WS)

    in_maps = []
    for c in range(N_CORES):
        h0, h1 = 2 * c, 2 * c + 1
        qh = [Wq[h * D_QK:(h + 1) * D_QK] for h in (h0, h1)]
        kvh = [Wkv[h * (D_NOPE + D_V):(h + 1) * (D_NOPE + D_V)] for h in (h0, h1)]
        qpe0, qpe1 = qh[0][D_NOPE:], qh[1][D_NOPE:]
        W_all = np.concatenate([
            qh[0][:D_NOPE], qh[1][:D_NOPE],
            qpe0[ev], qpe1[ev], qpe0[od], qpe1[od],
            kvh[0][:D_NOPE], kvh[1][:D_NOPE],
            wkpe[ev], wkpe[od],
            kvh[0][D_NOPE:], kvh[1][D_NOPE:],
        ], axis=0)                                               # (960, H)
        W8, Wr8 = _pack_contract(np.ascontiguousarray(W_all.T) * WS)
        wo_h = np.asarray(wo)[:, c * HPC * D_V:(c + 1) * HPC * D_V]   # (H, 256)
        woR = np.ascontiguousarray(wo_h.T).astype(BF16)          # (256, H) bf16

        in_maps.append({
            "hT8": hT8, "hTr8": hTr8,
            "wqa8": wqa8,
            "wkva8": wkva8,
            "W8": W8, "Wr8": Wr8,
            "woR": woR,
            "csq1": csq1, "csq2": csq2, "csk1": csk1, "csk2": csk2,
        })
    return in_maps


def _build():
    nc = bacc.Bacc()

    hT8 = nc.dram_tensor("hT8", [128, KCP, 2, TOK], F8, kind="ExternalInput")
    hTr8 = nc.dram_tensor("hTr8", [128, KCP, 2, TOK], F8, kind="ExternalInput")
    hTs8 = nc.dram_tensor("hTs8", [128, KCP, 2, TOKS], F8, kind="ExternalInput")
    wqa8 = nc.dram_tensor("wqa8", [128, KCP, 2, Q_LORA], F8, kind="ExternalInput")
    wkva8 = nc.dram_tensor("wkva8", [128, KCP, 2, KV_LORA], F8, kind="ExternalInput")
    W8d = nc.dram_tensor("W8", [128, KCP, 2, NPROJ], F8, kind="ExternalInput")
    Wr8d = nc.dram_tensor("Wr8", [128, KCP, 2, NPROJ], F8, kind="ExternalInput")
    woRd = nc.dram_tensor("woR", [HPC * D_V, H], BF, kind="ExternalInput")
    csq1d = nc.dram_tensor("csq1", [128, TOK], BF, kind="ExternalInput")
    csq2d = nc.dram_tensor("csq2", [128, TOK], BF, kind="ExternalInput")
    csk1d = nc.dram_tensor("csk1", [64, TOK], BF, kind="ExternalInput")
    csk2d = nc.dram_tensor("csk2", [64, TOK], BF, kind="ExternalInput")
    out = nc.dram_tensor("out", [TOK, H], BF, kind="ExternalOutput")

    AF = mybir.ActivationFunctionType
    OP = mybir.AluOpType

    with TileContext(nc) as tc:
        with tc.tile_pool(name="cst", bufs=1) as cst, \
             tc.tile_pool(name="ccp", bufs=1, space="DRAM") as ccp:

            ones_row = cst.tile([1, 128], BF)
            ones_col = cst.tile([128, 1], BF)
            eps_col = cst.tile([128, 1], F32)
            shift_col = cst.tile([128, 1], F32)
            nc.vector.memset(ones_row[:], 1.0)
            nc.vector.memset(ones_col[:], 1.0)
            nc.vector.memset(eps_col[:], EPS_SC)
            nc.vector.memset(shift_col[:], -SHIFT)
            cc_in = ccp.tile([1, 2 * TOKS], BF)
            cc_out = ccp.tile([N_CORES, 2 * TOKS], BF)

            with tc.tile_pool(name="acts", bufs=1) as acts:

                qn = [acts.tile([128, TOK], BF, tag=f"qn{h}", name=f"qn{h}")
                      for h in range(HPC)]
                qpe = [acts.tile([64, TOK], BF, tag=f"qpe{h}", name=f"qpe{h}")
                       for h in range(HPC)]
                kn = [acts.tile([128, TOK], BF, tag=f"kn{h}", name=f"kn{h}")
                      for h in range(HPC)]
                kpe = acts.tile([64, TOK], BF, tag="kpe", name="kpe")
                vnat = [acts.tile([128, HPC * D_V], BF, tag=f"v{i}", name=f"v{i}")
                        for i in range(TOK // 128)]

                # ---------------- phase 0: rms scalars ---------------------
                with tc.tile_pool(name="p0w", bufs=1) as p0w, \
                     tc.tile_pool(name="p0ps", bufs=2, space="PSUM") as p0ps, \
                     tc.tile_pool(name="p0sb", bufs=2) as p0sb:

                    hs_t = p0w.tile([128, KCP, 2, TOKS], F8, name="hs")
                    nc.gpsimd.dma_start(hs_t[:], hTs8[:])
                    wqa_t = p0w.tile([128, KCP, 2, Q_LORA], F8, name="wqa")
                    nc.gpsimd.dma_start(wqa_t[:], wqa8[:])
                    wkva_t = p0w.tile([128, KCP, 2, KV_LORA], F8, name="wkva")
                    nc.gpsimd.dma_start(wkva_t[:], wkva8[:])

                    for tb in range(TOKS // 128):
                        tsl = slice(tb * 128, (tb + 1) * 128)
                        ss_q = p0sb.tile([128, 1], F32, tag="ssq")
                        ss_kv = p0sb.tile([128, 1], F32, tag="sskv")
                        scratch = p0sb.tile([128, 512], F32, tag="scr")
                        for nb3 in range(Q_LORA // 512):
                            ps = p0ps.tile([128, 512], F32, tag="p0ps")
                            for k in range(KCP):
                                nc.tensor.matmul(
                                    ps[:], lhsT=hs_t[:, k, :, tsl],
                                    rhs=wqa_t[:, k, :, nb3 * 512:(nb3 + 1) * 512],
                                    start=(k == 0), stop=(k == KCP - 1),
                                    perf_mode=DR)
                            ssp = p0sb.tile([128, 1], F32, tag=f"ssp{nb3}",
                                            name=f"ssp{nb3}")
                            nc.scalar.activation(scratch[:], ps[:], AF.Square,
                                                 accum_out=ssp[:])
                            if nb3 == 0:
                                nc.vector.tensor_copy(ss_q[:], ssp[:])
                            else:
                                nc.vector.tensor_add(ss_q[:], ss_q[:], ssp[:])
                        ps = p0ps.tile([128, 512], F32, tag="p0ps")
                        for k in range(KCP):
                            nc.tensor.matmul(
                                ps[:], lhsT=hs_t[:, k, :, tsl], rhs=wkva_t[:, k, :, :],
                                start=(k == 0), stop=(k == KCP - 1), perf_mode=DR)
                        nc.scalar.activation(scratch[:], ps[:], AF.Square,
                                             accum_out=ss_kv[:])

                        # rms (x 2^14); reciprocal gives inv_rms x 2^-14,
                        # which also cancels the fp8 input/weight scales.
                        rms_q = p0sb.tile([128, 1], F32, tag="rmsq")
                        rms_kv = p0sb.tile([128, 1], F32, tag="rmskv")
                        nc.scalar.activation(rms_q[:], ss_q[:], AF.Sqrt,
                                             bias=eps_col[:], scale=1.0 / Q_LORA)
                        nc.scalar.activation(rms_kv[:], ss_kv[:], AF.Sqrt,
                                             bias=eps_col[:], scale=1.0 / KV_LORA)
                        inv_q = p0sb.tile([128, 1], BF, tag="invq")
                        inv_kv = p0sb.tile([128, 1], BF, tag="invkv")
                        with nc.allow_low_precision(
                                reason="inv_rms scalars; bf16 is a 0.2% "
                                "uniform per-token scale"):
                            nc.vector.reciprocal(inv_q[:], rms_q[:])
                            nc.vector.reciprocal(inv_kv[:], rms_kv[:])
                        nc.gpsimd.dma_start(cc_in[0, tsl], inv_q[:])
                        nc.gpsimd.dma_start(
                            cc_in[0, TOKS + tb * 128:TOKS + (tb + 1) * 128],
                            inv_kv[:])

                nc.gpsimd.collective_compute(
                    "AllGather", OP.bypass,
                    replica_groups=[list(range(N_CORES))],
                    ins=[cc_in.opt()], outs=[cc_out.opt()])

                # ---------------- phase 1: fused projections ----------------
                with tc.tile_pool(name="rowp", bufs=1) as rowp, \
                     tc.tile_pool(name="p1w", bufs=1) as p1w, \
                     tc.tile_pool(name="csp", bufs=1) as csp, \
                     tc.tile_pool(name="hp", bufs=2) as hp, \
                     tc.tile_pool(name="p1ps", bufs=1, space="PSUM") as p1ps, \
                     tc.tile_pool(name="p1vps", bufs=1, space="PSUM") as p1vps, \
                     tc.tile_pool(name="p1sb", bufs=1) as p1sb:

                    w8_t = p1w.tile([128, KCP, 2, NPROJ], F8, name="w8")
                    nc.gpsimd.dma_start(w8_t[:], W8d[:])
                    wr8_t = p1w.tile([128, KCP, 2, NPROJ], F8, name="wr8")
                    nc.gpsimd.dma_start(wr8_t[:], Wr8d[:])
                    csq1_t = csp.tile([128, TOK], BF, name="csq1")
                    csq2_t = csp.tile([128, TOK], BF, name="csq2")
                    nc.gpsimd.dma_start(csq1_t[:], csq1d[:])
                    nc.gpsimd.dma_start(csq2_t[:], csq2d[:])
                    csk1_t = csp.tile([64, TOK], BF, name="csk1")
                    csk2_t = csp.tile([64, TOK], BF, name="csk2")
                    nc.gpsimd.dma_start(csk1_t[:], csk1d[:])
                    nc.gpsimd.dma_start(csk2_t[:], csk2d[:])

                    inv_q_row = rowp.tile([1, N_CORES, TOKS], BF, name="ivqr")
                    inv_kv_row = rowp.tile([1, N_CORES, TOKS], BF, name="ivkr")
                    nc.gpsimd.dma_start(inv_q_row[:], cc_out[:, 0:TOKS])
                    nc.gpsimd.dma_start(inv_kv_row[:], cc_out[:, TOKS:2 * TOKS])

                    for nb in range(NB):
                        tsl = slice(nb * 512, (nb + 1) * 512)
                        ht = hp.tile([128, KCP, 2, 512], F8, tag="ht", name="ht")
                        nc.gpsimd.dma_start(ht[:], hT8[:, :, :, tsl])
                        htr = hp.tile([128, KCP, 2, 512], F8, tag="htr", name="htr")
                        nc.gpsimd.dma_start(htr[:], hTr8[:, :, :, tsl])

                        ps_feat = [p1ps.tile([128, 512], F32, tag=f"pf{mb}",
                                             name=f"pf{mb}") for mb in range(5)]
                        ps_feat.append(p1ps.tile([64, 512], F32, tag="pf5",
                                                 name="pf5"))
                        ps_v = [p1vps.tile([128, 2, 256], F32, tag=f"pv{i}",
                                           name=f"pv{i}") for i in range(2)]
                        # 3-term fp8 residual expansion of h @ W_all
                        terms = [(w8_t, ht), (w8_t, htr), (wr8_t, ht)]
                        nterm = len(terms)
                        for ti, (wt, hh) in enumerate(terms):
                            first = ti == 0
                            last = ti == nterm - 1
                            for k in range(KCP):
                                for mb in range(6):
                                    mrows = 64 if mb == 5 else 128
                                    nc.tensor.matmul(
                                        ps_feat[mb][:],
                                        lhsT=wt[:, k, :, mb * 128:mb * 128 + mrows],
                                        rhs=hh[:, k, :, :],
                                        start=(first and k == 0),
                                        stop=(last and k == KCP - 1),
                                        perf_mode=DR)
                                for sb4 in range(4):
                                    nc.tensor.matmul(
                                        ps_v[sb4 // 2][:, sb4 % 2, :],
                                        lhsT=hh[:, k, :, sb4 * 128:(sb4 + 1) * 128],
                                        rhs=wt[:, k, :, 704:960],
                                        start=(first and k == 0 and sb4 % 2 == 0),
                                        stop=(last and k == KCP - 1 and sb4 % 2 == 1),
                                        perf_mode=DR)

                        bq_t = p1sb.tile([128, 512], BF, tag="bq", name="bq")
                        nc.gpsimd.partition_broadcast(
                            bq_t[:], inv_q_row[0:1, nb, 0:TOKS])
                        bkv_t = p1sb.tile([128, 512], BF, tag="bkv", name="bkv")
                        nc.gpsimd.partition_broadcast(
                            bkv_t[:], inv_kv_row[0:1, nb, 0:TOKS])
                        bq = bq_t[:]
                        bkv = bkv_t[:]
                        # q/k_nope evictions: PSUM readers must be DVE or ACT
                        nc.vector.tensor_mul(qn[0][:, tsl], ps_feat[0][:], bq)
                        nc.vector.tensor_mul(qn[1][:, tsl], ps_feat[1][:], bq)
                        nc.vector.tensor_mul(kn[0][:, tsl], ps_feat[3][:], bkv)
                        nc.vector.tensor_mul(kn[1][:, tsl], ps_feat[4][:], bkv)

                        # v eviction on ACT: per-token (partition) inv scale
                        for sb4 in range(4):
                            tm = nb * 4 + sb4
                            ivc = p1sb.tile([128, 1], F32, tag="ivc", name="ivc")
                            nc.gpsimd.dma_start(
                                ivc[:],
                                cc_out[tm // 4, TOKS + (tm % 4) * 128:
                                       TOKS + (tm % 4) * 128 + 128])
                            nc.scalar.activation(
                                vnat[tm][:], ps_v[sb4 // 2][:, sb4 % 2, :],
                                AF.Copy, scale=ivc[:])

                        # rope q_pe stack [E0 E1 O0 O1] (x inv_q); muls on
                        # DVE (PSUM reads), add/sub on Pool (SBUF only)
                        tq = p1sb.tile([128, 512], F32, tag="tq", name="tq")
                        nc.vector.tensor_mul(tq[:], ps_feat[2][:], bq)
                        m1a = p1sb.tile([64, 512], F32, tag="m1a", name="m1a")
                        m1b = p1sb.tile([64, 512], F32, tag="m1b", name="m1b")
                        m2a = p1sb.tile([64, 512], F32, tag="m2a", name="m2a")
                        m2b = p1sb.tile([64, 512], F32, tag="m2b", name="m2b")
                        nc.vector.tensor_mul(m1a[:], tq[0:64, :], csq1_t[0:64, tsl])
                        nc.vector.tensor_mul(m1b[:], tq[64:128, :], csq1_t[64:128, tsl])
                        nc.vector.tensor_mul(m2a[:], tq[0:64, :], csq2_t[0:64, tsl])
                        nc.vector.tensor_mul(m2b[:], tq[64:128, :], csq2_t[64:128, tsl])
                        nc.gpsimd.tensor_sub(qpe[0][0:32, tsl],
                                             m1a[0:32, :], m1b[0:32, :])
                        nc.gpsimd.tensor_add(qpe[0][32:64, tsl],
                                             m2a[0:32, :], m2b[0:32, :])
                        nc.gpsimd.tensor_sub(qpe[1][0:32, tsl],
                                             m1a[32:64, :], m1b[32:64, :])
                        nc.gpsimd.tensor_add(qpe[1][32:64, tsl],
                                             m2a[32:64, :], m2b[32:64, :])

                        # rope k_pe stack [E O] (descale via tables)
                        mka = p1sb.tile([32, 512], F32, tag="mka", name="mka")
                        mkb = p1sb.tile([32, 512], F32, tag="mkb", name="mkb")
                        mkc = p1sb.tile([32, 512], F32, tag="mkc", name="mkc")
                        mkd = p1sb.tile([32, 512], F32, tag="mkd", name="mkd")
                        nc.vector.tensor_mul(mka[:], ps_feat[5][0:32, :],
                                             csk1_t[0:32, tsl])
                        nc.vector.tensor_mul(mkb[:], ps_feat[5][32:64, :],
                                             csk1_t[32:64, tsl])
                        nc.vector.tensor_mul(mkc[:], ps_feat[5][0:32, :],
                                             csk2_t[0:32, tsl])
                        nc.vector.tensor_mul(mkd[:], ps_feat[5][32:64, :],
                                             csk2_t[32:64, tsl])
                        nc.gpsimd.tensor_sub(kpe[0:32, tsl], mka[:], mkb[:])
                        nc.gpsimd.tensor_add(kpe[32:64, tsl], mkc[:], mkd[:])

                # ---------------- phase 2+3: attention + wo ------------------
                with tc.tile_pool(name="wop", bufs=1) as wop, \
                     tc.tile_pool(name="sps", bufs=2, space="PSUM") as sps, \
                     tc.tile_pool(name="ops", bufs=2, space="PSUM") as ops, \
                     tc.tile_pool(name="dps", bufs=1, space="PSUM") as dps, \
                     tc.tile_pool(name="wps", bufs=2, space="PSUM") as wps, \
                     tc.tile_pool(name="esb", bufs=4) as esb, \
                     tc.tile_pool(name="otp", bufs=2) as otp, \
                     tc.tile_pool(name="osb", bufs=2) as osb:

                    wo_t = []
                    for i in range(2):
                        t = wop.tile([128, H], BF, tag=f"wot{i}", name=f"wot{i}")
                        nc.gpsimd.dma_start(t[:], woRd[i * 128:(i + 1) * 128, :])
                        wo_t.append(t)

                    for b in range(B):
                        outT = [otp.tile([128, S], BF, tag=f"outT{h}",
                                         name=f"outT{h}") for h in range(HPC)]
                        for h in range(HPC):
                            for qb in range(S // 512):
                                qsl = slice(b * S + qb * 512, b * S + qb * 512 + 512)
                                osl = slice(qb * 512, qb * 512 + 512)
                                ps_o = ops.tile([128, 512], F32, tag="ps_o",
                                                name="ps_o")
                                ps_d = dps.tile([1, 512], F32, tag="ps_d",
                                                name="ps_d")
                                for kb in range(S // 128):
                                    ksl = slice(b * S + kb * 128,
                                                b * S + kb * 128 + 128)
                                    ps_s = sps.tile([128, 512], F32,
                                                    tag="ps_s", name="ps_s")
                                    nc.tensor.matmul(
                                        ps_s[:], lhsT=kn[h][:, ksl],
                                        rhs=qn[h][:, qsl],
                                        start=True, stop=False)
                                    nc.tensor.matmul(
                                        ps_s[:], lhsT=kpe[:, ksl],
                                        rhs=qpe[h][:, qsl],
                                        start=False, stop=True)
                                    ep = esb.tile([128, 512], BF, tag="ep",
                                                  name="ep")
                                    nc.scalar.activation(
                                        ep[:], ps_s[:], AF.Exp,
                                        bias=shift_col[:], scale=SCALE)
                                    tm = (b * S) // 128 + kb
                                    nc.tensor.matmul(
                                        ps_o[:],
                                        lhsT=vnat[tm][:, h * D_V:(h + 1) * D_V],
                                        rhs=ep[:],
                                        start=(kb == 0), stop=(kb == S // 128 - 1))
                                    if kb % 2 == 0:
                                        ep_prev = ep
                                    else:
                                        epsum = esb.tile([128, 512], BF,
                                                         tag="epsum", name="epsum")
                                        nc.vector.tensor_add(epsum[:], ep_prev[:],
                                                             ep[:])
                                        nc.tensor.matmul(
                                            ps_d[:], lhsT=ones_col[:], rhs=epsum[:],
                                            start=(kb == 1),
                                            stop=(kb == S // 128 - 1))
                                rec = esb.tile([1, 512], BF, tag="rec", name="rec")
                                with nc.allow_low_precision(
                                        reason="1/denom broadcast row, bf16 "
                                        "matmul rhs, 0.2% uniform per query"):
                                    nc.vector.reciprocal(rec[:], ps_d[:])
                                bc_sb = esb.tile([128, 512], BF, tag="bc_sb",
                                                 name="bc_sb")
                                nc.gpsimd.partition_broadcast(bc_sb[:], rec[:])
                                nc.vector.tensor_mul(outT[h][:, osl], ps_o[:],
                                                     bc_sb[:])

                        # wo for this batch (overlaps next batch attention)
                        for tmb in range(S // 128):
                            trow = b * S + tmb * 128
                            tksl = slice(tmb * 128, tmb * 128 + 128)
                            o_sb = osb.tile([128, H], BF, tag="o_sb", name="o_sb")
                            for hn in range(H // 512):
                                ps_w = wps.tile([128, 512], F32, tag="ps_w",
                                                name="ps_w")
                                for h in range(HPC):
                                    nc.tensor.matmul(
                                        ps_w[:], lhsT=outT[h][:, tksl],
                                        rhs=wo_t[h][:, hn * 512:(hn + 1) * 512],
                                        start=(h == 0), stop=(h == HPC - 1))
                                if hn % 2 == 0:
                                    nc.vector.tensor_copy(
                                        o_sb[:, hn * 512:(hn + 1) * 512], ps_w[:])
                                else:
                                    nc.scalar.activation(
                                        o_sb[:, hn * 512:(hn + 1) * 512], ps_w[:],
                                        AF.Copy)
                            nc.gpsimd.dma_start(out[trow:trow + 128, :], o_sb[:])

    nc.compile()
    return nc


_PROGRAM = None


def _get_program():
    global _PROGRAM
    if _PROGRAM is None:
        _PROGRAM = _build()
    return _PROGRAM


def kernel(hidden_states, wq_a, q_norm_w, wq_b, wkv_a, kv_norm_w, wkv_b, wo):
    nc = _get_program()
    in_maps = _host_prep(hidden_states, wq_a, q_norm_w, wq_b,
                         wkv_a, kv_norm_w, wkv_b, wo)
    for c in range(N_CORES):
        in_maps[c]["hTs8"] = np.ascontiguousarray(
            in_maps[c]["hT8"][:, :, :, c * TOKS:(c + 1) * TOKS])
    res = run_bass_kernel_spmd(nc, in_maps, list(range(N_CORES)))
    total = np.zeros((TOK, H), dtype=np.float32)
    for r in res.results:
        total += r["out"].astype(np.float32)
    return total.reshape(B, S, H)


# revision 35
# speedup vs baseline: 31091.8074x; 1.0996x over previous
"""MLA forward, sharded over 8 TRN2 NeuronCores.

Tensor-parallel over heads (2/core).  Host folds rmsnorm weights into the
B-projections and fuses A@B per head (rmsnorm's per-token scale commutes:
rmsnorm(x) @ Wb.T == (x @ (Wb*w).T) / rms(x)).

Precision strategy (fp8e4m3 flat-noise on zero-mean data is ~3-5% of the
output, so fp8 is only used where the rounding is provably recovered):
  - fused projection: fp8 DoubleRow with a 3-term residual expansion
      h*W ~= h8.W8 + hr8.W8 + h8.Wr8   (hr8/Wr8 = fp8 of the fp8 rounding
    error; leftover hr.Wr term is ~0.1%).  DoubleRow packs K=256 per
    instruction at 0.5 PE cycles/row, so 3 terms still beat bf16 by 25%.
  - phase-0 rms statistics: single-term fp8 DoubleRow (rms is a mean of
    1536 squares; fp8 noise averages to <0.2% there).
  - everything else (scores, exp, PV, denominator, wo) in bf16.

Phases:
  0  rms stats for the core's 512-token shard, AllGather of inv_rms.
  1  fused projection per 512-token block -> qn/qpe/kn/kpe feature-major
     bf16, v token-major bf16.  Evictions: DVE (q/k + q-rope muls),
     Pool (rope add/sub from SBUF), ACT (v via per-partition inv scale).
  2  scores^T per 128-k-block (bf16 nope K=128 + rope K=64 into one PSUM
     bank), exp on ACT (global shift 2.0), PV bf16, denominator via
     ones-matmuls accumulated in PSUM, normalize, then bf16 wo.
  host sums the 8 bf16 partial outputs (the "all-reduce after wo").
"""
import sys

sys.path.insert(0, "/opt/trn_rl_repo")

import numpy as np
import ml_dtypes

import concourse.mybir as mybir
from concourse import bacc
from concourse.tile import TileContext
from concourse.bass_utils import run_bass_kernel_spmd

NP8 = ml_dtypes.float8_e4m3
BF16 = ml_dtypes.bfloat16
F32 = mybir.dt.float32
BF = mybir.dt.bfloat16
F8 = mybir.dt.float8e4
DR = mybir.MatmulPerfMode.DoubleRow

B, S, H = 2, 2048, 2048
NH = 16
Q_LORA, KV_LORA = 1536, 512
D_NOPE, D_ROPE, D_V = 128, 64, 128
D_QK = D_NOPE + D_ROPE
SCALE = 1.0 / float(np.sqrt(D_QK))
EPS = 1e-6

N_CORES = 8
HPC = NH // N_CORES          # heads per core = 2
TOK = B * S                  # 4096
TOKS = TOK // N_CORES        # 512-token rms shard per core
KCP = H // 256               # 8 contraction PAIRS over hidden features
NB = TOK // 512              # 8 token blocks of 512

HS = 32.0                    # hidden fp8 scale (2^5)
WS = 512.0                   # weight fp8 scale (2^9)
EPS_SC = EPS * (HS * WS) ** 2    # eps * 2^28, for scaled sum-of-squares
SHIFT = 2.0                  # global softmax exp shift (softmax-invariant)

# W_all column layout (projection output features, per core):
#   [0:128) qn h0  [128:256) qn h1  [256:384) qpe E0 E1 O0 O1 (32 each)
#   [384:512) kn h0  [512:640) kn h1  [640:704) kpe E(32) O(32)
#   [704:960) v h0(128) v h1(128)
NPROJ = 960


def _pack_contract(a):
    """(H, F) f32 -> ([128, KCP, 2, F] fp8 main, same-shape fp8 residual)."""
    hdim, f = a.shape
    assert hdim == H
    p = np.ascontiguousarray(a.reshape(KCP, 2, 128, f).transpose(2, 0, 1, 3))
    m = p.astype(NP8)
    r = (p - m.astype(np.float32)).astype(NP8)
    return m, r


def _host_tables():
    inv = 1.0 / (10000.0 ** (np.arange(0, D_ROPE, 2, dtype=np.float32) / D_ROPE))
    t = np.arange(S, dtype=np.float32)
    f = np.outer(t, inv)                       # (S, 32)
    cos = np.tile(np.cos(f).T, (1, B))         # (32, TOK), tokens b-major
    sin = np.tile(np.sin(f).T, (1, B))
    csq1 = np.concatenate([cos, cos, sin, sin], axis=0)   # (128, TOK)
    csq2 = np.concatenate([sin, sin, cos, cos], axis=0)
    kd = 1.0 / (HS * WS)                       # 2^-14 descale for k_pe
    csk1 = np.concatenate([cos, sin], axis=0) * kd        # (64, TOK)
    csk2 = np.concatenate([sin, cos], axis=0) * kd
    return [np.ascontiguousarray(x).astype(BF16) for x in (csq1, csq2, csk1, csk2)]


def _host_prep(hidden_states, wq_a, q_norm_w, wq_b, wkv_a, kv_norm_w, wkv_b, wo):
    hid = np.ascontiguousarray(
        np.asarray(hidden_states, dtype=np.float32).reshape(TOK, H))
    hT8, hTr8 = _pack_contract(np.ascontiguousarray(hid.T) * HS)

    wq_b_f = (np.asarray(wq_b) * np.asarray(q_norm_w)[None, :]).astype(np.float32)
    wkv_b_f = (np.asarray(wkv_b) * np.asarray(kv_norm_w)[None, :]).astype(np.float32)

    Wq = wq_b_f @ np.asarray(wq_a)                 # (NH*192, H)
    Wkv = wkv_b_f @ np.asarray(wkv_a)[:KV_LORA]    # (NH*256, H)
    wkpe = np.asarray(wkv_a)[KV_LORA:]             # (64, H)

    ev = np.arange(0, D_ROPE, 2)
    od = np.arange(1, D_ROPE, 2)
    csq1, csq2, csk1, csk2 = _host_tables()

    wqa8, _ = _pack_contract(np.asarray(wq_a).T.astype(np.float32) * WS)
    wkva8, _ = _pack_contract(np.asarray(wkv_a)[:KV_LORA].T.astype(np.float32) * WS)

    in_maps = []
    for c in range(N_CORES):
        h0, h1 = 2 * c, 2 * c + 1
        qh = [Wq[h * D_QK:(h + 1) * D_QK] for h in (h0, h1)]
        kvh = [Wkv[h * (D_NOPE + D_V):(h + 1) * (D_NOPE + D_V)] for h in (h0, h1)]
        qpe0, qpe1 = qh[0][D_NOPE:], qh[1][D_NOPE:]
        W_all = np.concatenate([
            qh[0][:D_NOPE], qh[1][:D_NOPE],
            qpe0[ev], qpe1[ev], qpe0[od], qpe1[od],
            kvh[0][:D_NOPE], kvh[1][:D_NOPE],
            wkpe[ev], wkpe[od],
            kvh[0][D_NOPE:], kvh[1][D_NOPE:],
        ], axis=0)                                               # (960, H)
        W8, Wr8 = _pack_contract(np.ascontiguousarray(W_all.T) * WS)
        wo_h = np.asarray(wo)[:, c * HPC * D_V:(c + 1) * HPC * D_V]   # (H, 256)
        woR = np.ascontiguousarray(wo_h.T).astype(BF16)          # (256, H) bf16

        in_maps.append({
            "hT8": hT8, "hTr8": hTr8,
            "wqa8": wqa8,
            "wkva8": wkva8,
            "W8": W8, "Wr8": Wr8,
            "woR": woR,
            "csq1": csq1, "csq2": csq2, "csk1": csk1, "csk2": csk2,
        })
    return in_maps


def _build():
    nc = bacc.Bacc()

    hT8 = nc.dram_tensor("hT8", [128, KCP, 2, TOK], F8, kind="ExternalInput")
    hTr8 = nc.dram_tensor("hTr8", [128, KCP, 2, TOK], F8, kind="ExternalInput")
    hTs8 = nc.dram_tensor("hTs8", [128, KCP, 2, TOKS], F8, kind="ExternalInput")
    wqa8 = nc.dram_tensor("wqa8", [128, KCP, 2, Q_LORA], F8, kind="ExternalInput")
    wkva8 = nc.dram_tensor("wkva8", [128, KCP, 2, KV_LORA], F8, kind="ExternalInput")
    W8d = nc.dram_tensor("W8", [128, KCP, 2, NPROJ], F8, kind="ExternalInput")
    Wr8d = nc.dram_tensor("Wr8", [128, KCP, 2, NPROJ], F8, kind="ExternalInput")
    woRd = nc.dram_tensor("woR", [HPC * D_V, H], BF, kind="ExternalInput")
    csq1d = nc.dram_tensor("csq1", [128, TOK], BF, kind="ExternalInput")
    csq2d = nc.dram_tensor("csq2", [128, TOK], BF, kind="ExternalInput")
    csk1d = nc.dram_tensor("csk1", [64, TOK], BF, kind="ExternalInput")
    csk2d = nc.dram_tensor("csk2", [64, TOK], BF, kind="ExternalInput")
    out = nc.dram_tensor("out", [TOK, H], BF, kind="ExternalOutput")

    AF = mybir.ActivationFunctionType
    OP = mybir.AluOpType

    with TileContext(nc) as tc:
        with tc.tile_pool(name="cst", bufs=1) as cst, \
             tc.tile_pool(name="ccp", bufs=1, space="DRAM") as ccp:

            ones_row = cst.tile([1, 128], BF)
            ones_col = cst.tile([128, 1], BF)
            eps_col = cst.tile([128, 1], F32)
            shift_col = cst.tile([128, 1], F32)
            nc.vector.memset(ones_row[:], 1.0)
            nc.vector.memset(ones_col[:], 1.0)
            nc.vector.memset(eps_col[:], EPS_SC)
            nc.vector.memset(shift_col[:], -SHIFT)
            cc_in = ccp.tile([1, 2 * TOKS], BF)
            cc_out = ccp.tile([N_CORES, 2 * TOKS], BF)

            with tc.tile_pool(name="acts", bufs=1) as acts:

                qn = [[acts.tile([128, S], BF, tag=f"qn{b}{h}", name=f"qn{b}{h}")
                       for h in range(HPC)] for b in range(B)]
                qpe = [[acts.tile([64, S], BF, tag=f"qpe{b}{h}",
                        name=f"qpe{b}{h}") for h in range(HPC)] for b in range(B)]
                kn = [[acts.tile([128, S], BF, tag=f"kn{b}{h}", name=f"kn{b}{h}")
                       for h in range(HPC)] for b in range(B)]
                kpe = [acts.tile([64, S], BF, tag=f"kpe{b}", name=f"kpe{b}")
                       for b in range(B)]
                vnat = [acts.tile([128, HPC * D_V], BF, tag=f"v{i}", name=f"v{i}")
                        for i in range(TOK // 128)]

                # phase-1 input pools open early so their DMAs overlap
                # phase-0 compute; closed before phase 2
                ph1_pools = [
                    tc.tile_pool(name="p1w", bufs=1),
                    tc.tile_pool(name="csp", bufs=1),
                    tc.tile_pool(name="hp", bufs=2),
                ]
                from contextlib import ExitStack
                _ph1 = ExitStack()
                p1w, csp, hp = (_ph1.enter_context(p) for p in ph1_pools)

                # ---------------- phase 0: rms scalars ---------------------
                with tc.tile_pool(name="p0w", bufs=1) as p0w, \
                     tc.tile_pool(name="p0wq", bufs=2) as p0wq, \
                     tc.tile_pool(name="p0ps", bufs=2, space="PSUM") as p0ps, \
                     tc.tile_pool(name="p0sb", bufs=2) as p0sb:

                    hs_t = p0w.tile([128, KCP, 2, TOKS], F8, name="hs")
                    nc.gpsimd.dma_start(hs_t[:], hTs8[:])
                    wkva_t = p0w.tile([128, KCP, 2, KV_LORA], F8, name="wkva")
                    nc.gpsimd.dma_start(wkva_t[:], wkva8[:])

                    # phase-1 inputs stream on the idle SYNC queue, ordered
                    # so the fold's first term can start earliest
                    w8_t = p1w.tile([128, KCP, 2, NPROJ], F8, name="w8")
                    nc.sync.dma_start(w8_t[:], W8d[:])
                    ht0 = hp.tile([128, KCP, 2, 512], F8, tag="ht", name="ht")
                    nc.sync.dma_start(ht0[:], hT8[:, :, :, 0:512])
                    htr0 = hp.tile([128, KCP, 2, 512], F8, tag="htr", name="htr")
                    nc.sync.dma_start(htr0[:], hTr8[:, :, :, 0:512])
                    wr8_t = p1w.tile([128, KCP, 2, NPROJ], F8, name="wr8")
                    nc.sync.dma_start(wr8_t[:], Wr8d[:])
                    csq1_t = csp.tile([128, TOK], BF, name="csq1")
                    csq2_t = csp.tile([128, TOK], BF, name="csq2")
                    nc.sync.dma_start(csq1_t[:], csq1d[:])
                    nc.sync.dma_start(csq2_t[:], csq2d[:])
                    csk1_t = csp.tile([64, TOK], BF, name="csk1")
                    csk2_t = csp.tile([64, TOK], BF, name="csk2")
                    nc.sync.dma_start(csk1_t[:], csk1d[:])
                    nc.sync.dma_start(csk2_t[:], csk2d[:])

                    ss_qs, ss_kvs = [], []
                    for tb in range(TOKS // 128):
                        tsl = slice(tb * 128, (tb + 1) * 128)
                        ss_kv = p0sb.tile([128, 1], F32, tag=f"sskv{tb}",
                                          name=f"sskv{tb}")
                        scratch = p0sb.tile([128, 512], BF, tag="scr")
                        ps = p0ps.tile([128, 512], F32, tag="p0ps")
                        for k in range(KCP):
                            nc.tensor.matmul(
                                ps[:], lhsT=hs_t[:, k, :, tsl], rhs=wkva_t[:, k, :, :],
                                start=(k == 0), stop=(k == KCP - 1), perf_mode=DR)
                        nc.scalar.activation(scratch[:], ps[:], AF.Square,
                                             accum_out=ss_kv[:])
                        ss_kvs.append(ss_kv)
                        ss_qs.append(p0sb.tile([128, 1], F32, tag=f"ssq{tb}",
                                               name=f"ssq{tb}"))
                    for nb6 in range(Q_LORA // 256):
                        wqa_t = p0wq.tile([128, KCP, 2, 256], F8, tag="wqac",
                                          name="wqac")
                        nc.gpsimd.dma_start(
                            wqa_t[:], wqa8[:, :, :, nb6 * 256:(nb6 + 1) * 256])
                        for tb in range(TOKS // 128):
                            tsl = slice(tb * 128, (tb + 1) * 128)
                            scratch = p0sb.tile([128, 256], BF, tag="scr")
                            ps = p0ps.tile([128, 256], F32, tag="p0psq")
                            for k in range(KCP):
                                nc.tensor.matmul(
                                    ps[:], lhsT=hs_t[:, k, :, tsl],
                                    rhs=wqa_t[:, k, :, :],
                                    start=(k == 0), stop=(k == KCP - 1),
                                    perf_mode=DR)
                            ssp = p0sb.tile([128, 1], F32, tag=f"ssp{tb}",
                                            name=f"ssp{tb}")
                            nc.scalar.activation(scratch[:], ps[:], AF.Square,
                                                 accum_out=ssp[:])
                            if nb6 == 0:
                                nc.vector.tensor_copy(ss_qs[tb][:], ssp[:])
                            else:
                                nc.vector.tensor_add(ss_qs[tb][:], ss_qs[tb][:],
                                                     ssp[:])
                    for tb in range(TOKS // 128):
                        tsl = slice(tb * 128, (tb + 1) * 128)
                        ss_q, ss_kv = ss_qs[tb], ss_kvs[tb]

                        # rms (x 2^14); reciprocal gives inv_rms x 2^-14,
                        # which also cancels the fp8 input/weight scales.
                        rms_q = p0sb.tile([128, 1], F32, tag="rmsq")
                        rms_kv = p0sb.tile([128, 1], F32, tag="rmskv")
                        nc.scalar.activation(rms_q[:], ss_q[:], AF.Sqrt,
                                             bias=eps_col[:], scale=1.0 / Q_LORA)
                        nc.scalar.activation(rms_kv[:], ss_kv[:], AF.Sqrt,
                                             bias=eps_col[:], scale=1.0 / KV_LORA)
                        inv_q = p0sb.tile([128, 1], BF, tag="invq")
                        inv_kv = p0sb.tile([128, 1], BF, tag="invkv")
                        with nc.allow_low_precision(
                                reason="inv_rms scalars; bf16 is a 0.2% "
                                "uniform per-token scale"):
                            nc.vector.reciprocal(inv_q[:], rms_q[:])
                            nc.vector.reciprocal(inv_kv[:], rms_kv[:])
                        nc.gpsimd.dma_start(cc_in[0, tsl], inv_q[:])
                        nc.gpsimd.dma_start(
                            cc_in[0, TOKS + tb * 128:TOKS + (tb + 1) * 128],
                            inv_kv[:])

                nc.gpsimd.collective_compute(
                    "AllGather", OP.bypass,
                    replica_groups=[list(range(N_CORES))],
                    ins=[cc_in.opt()], outs=[cc_out.opt()])

                # ---------------- phase 1: fused projections ----------------
                with tc.tile_pool(name="p1ps", bufs=1, space="PSUM") as p1ps, \
                     tc.tile_pool(name="p1vps", bufs=1, space="PSUM") as p1vps, \
                     tc.tile_pool(name="p1sb", bufs=1) as p1sb:

                    for nb in range(NB):
                        tsl = slice(nb * 512, (nb + 1) * 512)
                        if nb == 0:
                            ht, htr = ht0, htr0
                        else:
                            ht = hp.tile([128, KCP, 2, 512], F8, tag="ht", name="ht")
                            nc.sync.dma_start(ht[:], hT8[:, :, :, tsl])
                            htr = hp.tile([128, KCP, 2, 512], F8, tag="htr",
                                          name="htr")
                            nc.sync.dma_start(htr[:], hTr8[:, :, :, tsl])

                        ps_feat = [p1ps.tile([128, 512], F32, tag=f"pf{mb}",
                                             name=f"pf{mb}") for mb in range(5)]
                        ps_feat.append(p1ps.tile([64, 512], F32, tag="pf5",
                                                 name="pf5"))
                        ps_v = [p1vps.tile([128, 2, 256], F32, tag=f"pv{i}",
                                           name=f"pv{i}") for i in range(2)]
                        # 3-term fp8 residual expansion of h @ W_all
                        terms = [(w8_t, ht), (w8_t, htr), (wr8_t, ht)]
                        nterm = len(terms)
                        for ti, (wt, hh) in enumerate(terms):
                            first = ti == 0
                            last = ti == nterm - 1
                            for k in range(KCP):
                                for mb in range(6):
                                    mrows = 64 if mb == 5 else 128
                                    nc.tensor.matmul(
                                        ps_feat[mb][:],
                                        lhsT=wt[:, k, :, mb * 128:mb * 128 + mrows],
                                        rhs=hh[:, k, :, :],
                                        start=(first and k == 0),
                                        stop=(last and k == KCP - 1),
                                        perf_mode=DR)
                                for sb4 in range(4):
                                    nc.tensor.matmul(
                                        ps_v[sb4 // 2][:, sb4 % 2, :],
                                        lhsT=hh[:, k, :, sb4 * 128:(sb4 + 1) * 128],
                                        rhs=wt[:, k, :, 704:960],
                                        start=(first and k == 0 and sb4 % 2 == 0),
                                        stop=(last and k == KCP - 1 and sb4 % 2 == 1),
                                        perf_mode=DR)

                        row_q = p1sb.tile([1, 512], BF, tag="rowq", name="rowq")
                        nc.gpsimd.dma_start(row_q[:], cc_out[nb:nb + 1, 0:TOKS])
                        row_kv = p1sb.tile([1, 512], BF, tag="rowkv", name="rowkv")
                        nc.gpsimd.dma_start(row_kv[:],
                                            cc_out[nb:nb + 1, TOKS:2 * TOKS])
                        bq_t = p1sb.tile([128, 512], BF, tag="bq", name="bq")
                        nc.gpsimd.partition_broadcast(bq_t[:], row_q[:])
                        bkv_t = p1sb.tile([128, 512], BF, tag="bkv", name="bkv")
                        nc.gpsimd.partition_broadcast(bkv_t[:], row_kv[:])
                        bq = bq_t[:]
                        bkv = bkv_t[:]
                        bb = nb // (NB // B)
                        bsl = slice((nb % (NB // B)) * 512,
                                    (nb % (NB // B)) * 512 + 512)
                        # q/k_nope evictions: PSUM readers must be DVE or ACT
                        nc.vector.tensor_mul(qn[bb][0][:, bsl], ps_feat[0][:], bq)
                        nc.vector.tensor_mul(qn[bb][1][:, bsl], ps_feat[1][:], bq)
                        nc.vector.tensor_mul(kn[bb][0][:, bsl], ps_feat[3][:], bkv)
                        nc.vector.tensor_mul(kn[bb][1][:, bsl], ps_feat[4][:], bkv)

                        # v eviction on ACT: per-token (partition) inv scale
                        for sb4 in range(4):
                            tm = nb * 4 + sb4
                            ivc = p1sb.tile([128, 1], F32, tag="ivc", name="ivc")
                            nc.gpsimd.dma_start(
                                ivc[:],
                                cc_out[tm // 4, TOKS + (tm % 4) * 128:
                                       TOKS + (tm % 4) * 128 + 128])
                            nc.scalar.activation(
                                vnat[tm][:], ps_v[sb4 // 2][:, sb4 % 2, :],
                                AF.Copy, scale=ivc[:])

                        # rope q_pe stack [E0 E1 O0 O1] (x inv_q); muls on
                        # DVE (PSUM reads), add/sub on Pool (SBUF only)
                        tq = p1sb.tile([128, 512], BF, tag="tq", name="tq")
                        nc.vector.tensor_mul(tq[:], ps_feat[2][:], bq)
                        m1a = p1sb.tile([64, 512], BF, tag="m1a", name="m1a")
                        m1b = p1sb.tile([64, 512], BF, tag="m1b", name="m1b")
                        m2a = p1sb.tile([64, 512], BF, tag="m2a", name="m2a")
                        m2b = p1sb.tile([64, 512], BF, tag="m2b", name="m2b")
                        nc.vector.tensor_mul(m1a[:], tq[0:64, :], csq1_t[0:64, tsl])
                        nc.vector.tensor_mul(m1b[:], tq[64:128, :], csq1_t[64:128, tsl])
                        nc.vector.tensor_mul(m2a[:], tq[0:64, :], csq2_t[0:64, tsl])
                        nc.vector.tensor_mul(m2b[:], tq[64:128, :], csq2_t[64:128, tsl])
                        nc.gpsimd.tensor_sub(qpe[bb][0][0:32, bsl],
                                             m1a[0:32, :], m1b[0:32, :])
                        nc.gpsimd.tensor_add(qpe[bb][0][32:64, bsl],
                                             m2a[0:32, :], m2b[0:32, :])
                        nc.gpsimd.tensor_sub(qpe[bb][1][0:32, bsl],
                                             m1a[32:64, :], m1b[32:64, :])
                        nc.gpsimd.tensor_add(qpe[bb][1][32:64, bsl],
                                             m2a[32:64, :], m2b[32:64, :])

                        # rope k_pe stack [E O] (descale via tables)
                        mka = p1sb.tile([32, 512], BF, tag="mka", name="mka")
                        mkb = p1sb.tile([32, 512], BF, tag="mkb", name="mkb")
                        mkc = p1sb.tile([32, 512], BF, tag="mkc", name="mkc")
                        mkd = p1sb.tile([32, 512], BF, tag="mkd", name="mkd")
                        nc.vector.tensor_mul(mka[:], ps_feat[5][0:32, :],
                                             csk1_t[0:32, tsl])
                        nc.vector.tensor_mul(mkb[:], ps_feat[5][32:64, :],
                                             csk1_t[32:64, tsl])
                        nc.vector.tensor_mul(mkc[:], ps_feat[5][0:32, :],
                                             csk2_t[0:32, tsl])
                        nc.vector.tensor_mul(mkd[:], ps_feat[5][32:64, :],
                                             csk2_t[32:64, tsl])
                        nc.gpsimd.tensor_sub(kpe[bb][0:32, bsl], mka[:], mkb[:])
                        nc.gpsimd.tensor_add(kpe[bb][32:64, bsl], mkc[:], mkd[:])

                _ph1.close()

                # ---------------- phase 2+3: attention + wo ------------------
                with tc.tile_pool(name="wop", bufs=1) as wop, \
                     tc.tile_pool(name="sps", bufs=2, space="PSUM") as sps, \
                     tc.tile_pool(name="ops", bufs=2, space="PSUM") as ops, \
                     tc.tile_pool(name="dps", bufs=1, space="PSUM") as dps, \
                     tc.tile_pool(name="wps", bufs=2, space="PSUM") as wps, \
                     tc.tile_pool(name="esb", bufs=4) as esb, \
                     tc.tile_pool(name="otp", bufs=2) as otp, \
                     tc.tile_pool(name="osb", bufs=2) as osb:

                    wo_t = []
                    for i in range(2):
                        t = wop.tile([128, H], BF, tag=f"wot{i}", name=f"wot{i}")
                        nc.sync.dma_start(t[:], woRd[i * 128:(i + 1) * 128, :])
                        wo_t.append(t)

                    for b in range(B):
                        outT = [otp.tile([128, S], BF, tag=f"outT{h}",
                                         name=f"outT{h}") for h in range(HPC)]
                        for h in range(HPC):
                            for qb in range(S // 512):
                                qsl = slice(qb * 512, qb * 512 + 512)
                                osl = slice(qb * 512, qb * 512 + 512)
                                ps_o = ops.tile([128, 512], F32, tag="ps_o",
                                                name="ps_o")
                                ps_d = dps.tile([1, 512], F32, tag="ps_d",
                                                name="ps_d")
                                for kb in range(S // 128):
                                    ksl = slice(kb * 128, kb * 128 + 128)
                                    ps_s = sps.tile([128, 512], F32,
                                                    tag="ps_s", name="ps_s")
                                    nc.tensor.matmul(
                                        ps_s[:], lhsT=kn[b][h][:, ksl],
                                        rhs=qn[b][h][:, qsl],
                                        start=True, stop=False)
                                    nc.tensor.matmul(
                                        ps_s[:], lhsT=kpe[b][:, ksl],
                                        rhs=qpe[b][h][:, qsl],
                                        start=False, stop=True)
                                    ep = esb.tile([128, 512], BF, tag="ep",
                                                  name="ep")
                                    nc.scalar.activation(
                                        ep[:], ps_s[:], AF.Exp,
                                        bias=shift_col[:], scale=SCALE)
                                    tm = (b * S) // 128 + kb
                                    nc.tensor.matmul(
                                        ps_o[:],
                                        lhsT=vnat[tm][:, h * D_V:(h + 1) * D_V],
                                        rhs=ep[:],
                                        start=(kb == 0), stop=(kb == S // 128 - 1))
                                    if kb % 2 == 0:
                                        ep_prev = ep
                                    elif kb == 1:
                                        dacc = esb.tile([128, 512], BF,
                                                        tag="dacc", name="dacc")
                                        nc.vector.tensor_add(dacc[:], ep_prev[:],
                                                             ep[:])
                                    else:
                                        epsum = esb.tile([128, 512], BF,
                                                         tag="epsum", name="epsum")
                                        nc.vector.tensor_add(epsum[:], ep_prev[:],
                                                             ep[:])
                                        nc.vector.tensor_add(dacc[:], dacc[:],
                                                             epsum[:])
                                nc.tensor.matmul(ps_d[:], lhsT=ones_col[:],
                                                 rhs=dacc[:], start=True, stop=True)
                                rec = esb.tile([1, 512], BF, tag="rec", name="rec")
                                with nc.allow_low_precision(
                                        reason="1/denom broadcast row, bf16 "
                                        "matmul rhs, 0.2% uniform per query"):
                                    nc.vector.reciprocal(rec[:], ps_d[:])
                                bc_sb = esb.tile([128, 512], BF, tag="bc_sb",
                                                 name="bc_sb")
                                nc.gpsimd.partition_broadcast(bc_sb[:], rec[:])
                                nc.vector.tensor_mul(outT[h][:, osl], ps_o[:],
                                                     bc_sb[:])

                        # wo for this batch (overlaps next batch attention)
                        for tmb in range(S // 128):
                            trow = b * S + tmb * 128
                            tksl = slice(tmb * 128, tmb * 128 + 128)
                            o_sb = osb.tile([128, H], BF, tag="o_sb", name="o_sb")
                            for hn in range(H // 512):
                                ps_w = wps.tile([128, 512], F32, tag="ps_w",
                                                name="ps_w")
                                for h in range(HPC):
                                    nc.tensor.matmul(
                                        ps_w[:], lhsT=outT[h][:, tksl],
                                        rhs=wo_t[h][:, hn * 512:(hn + 1) * 512],
                                        start=(h == 0), stop=(h == HPC - 1))
                                if hn % 2 == 0:
                                    nc.vector.tensor_copy(
                                        o_sb[:, hn * 512:(hn + 1) * 512], ps_w[:])
                                else:
                                    nc.scalar.activation(
                                        o_sb[:, hn * 512:(hn + 1) * 512], ps_w[:],
                                        AF.Copy)
                            nc.sync.dma_start(out[trow:trow + 128, :], o_sb[:])

    nc.compile()
    return nc


_PROGRAM = None


def _get_program():
    global _PROGRAM
    if _PROGRAM is None:
        _PROGRAM = _build()
    return _PROGRAM


def kernel(hidden_states, wq_a, q_norm_w, wq_b, wkv_a, kv_norm_w, wkv_b, wo):
    nc = _get_program()
    in_maps = _host_prep(hidden_states, wq_a, q_norm_w, wq_b,
                         wkv_a, kv_norm_w, wkv_b, wo)
    for c in range(N_CORES):
        in_maps[c]["hTs8"] = np.ascontiguousarray(
            in_maps[c]["hT8"][:, :, :, c * TOKS:(c + 1) * TOKS])
    res = run_bass_kernel_spmd(nc, in_maps, list(range(N_CORES)))
    total = np.zeros((TOK, H), dtype=np.float32)
    for r in res.results:
        total += r["out"].astype(np.float32)
    return total.reshape(B, S, H)
